# revision 1
# baseline (speedup 1.0000x reference)
"""MinimumErrorRateLoss on 8 Trainium2 NeuronCores.

Strategy: the loss is dominated by B = N*M = 4096 independent Levenshtein
edit-distance DPs (ref length R=256 vs hyp length H=288). We shard the
flattened pair dimension across the 8 cores (512 pairs/core), laid out as
128 SBUF partitions x 4 free-dim segments. On each core the DP runs as an
anti-diagonal wavefront: every anti-diagonal d is a pure elementwise update

    new[i] = min( min(prev[i], prev[i-1]) + 1, prev2[i-1] + neq[i] )

over cells i in [max(0,d-H), min(d,R)], which maps to 4 VectorEngine ops
(not_equal, add, min, fused scalar_tensor_tensor) with +-1-shifted access
patterns on three rotating diagonal buffers. Out-of-range reads resolve to
BIG guard values placed by one-time memsets (never overwritten thereafter).
The final softmax/mean reduction over 4096 floats is done on host.
"""

import numpy as np

N, M, R, H = 128, 32, 256, 288
NCORES = 8
P = 128          # SBUF partitions
SEG = 4          # segments per partition -> 512 pairs per core
BPC = P * SEG    # pairs per core
SSTRIDE = 260    # V-buffer slots per segment (slot = i + 2, i in 0..R)
HSTRIDE = 292    # flipped-hyp slots per segment (H + 1 pad, rounded up)
BIG = 30000.0

_CACHE = {}


def _build_program(r, h, sstride, hstride, reps=1):
    from contextlib import ExitStack

    import concourse.bass as bass
    import concourse.mybir as mybir

    nc = bass.Bass(
        "TRN2", target_bir_lowering=False, debug=False,
        detect_race_conditions=False,
    )
    dt = mybir.dt.float32
    # ref and hyp packed in one input so the load is a single DMA (walrus
    # here allows at most 2 sync commands per instruction).
    inp = nc.dram_tensor(
        "inp", [P, SEG, sstride + hstride], dt, kind="ExternalInput"
    ).ap()
    dist_out = nc.dram_tensor("dist", [P, SEG, 1], dt, kind="ExternalOutput").ap()
    AOT = mybir.AluOpType

    with ExitStack() as ctx:
        inpt = ctx.enter_context(
            nc.sbuf_tensor("inpt", [P, SEG, sstride + hstride], dt)
        )
        v = [
            ctx.enter_context(nc.sbuf_tensor(f"v{k}", [P, SEG, sstride], dt))
            for k in range(3)
        ]
        # two nq buffers (ping-pong by diagonal parity): neq(d+1) is emitted
        # before sb(d) consumes neq(d)'s output
        nqs = [
            ctx.enter_context(nc.sbuf_tensor(f"nq{k}", [P, SEG, sstride], dt))
            for k in range(2)
        ]
        sb = ctx.enter_context(nc.sbuf_tensor("sb", [P, SEG, sstride], dt))
        mb = ctx.enter_context(nc.sbuf_tensor("mb", [P, SEG, sstride], dt))
        outt = ctx.enter_context(nc.sbuf_tensor("outt", [P, SEG, 1], dt))
        dma_sem = ctx.enter_context(nc.semaphore("dma_sem"))
        vdone = ctx.enter_context(nc.semaphore("vdone"))
        dve_sem = ctx.enter_context(nc.semaphore("dve_sem"))
        block = ctx.enter_context(nc.Block())

        reft = inpt[:, :, :sstride]
        hypt = inpt[:, :, sstride:]
        vf = v[(r + h) % 3]

        # the NEFF may be executed more than once on one load: reset the
        # semaphores at the END of each run (after both DMAs completed) so
        # every execution starts from zero.
        @block.gpsimd
        def _(gpsimd):
            gpsimd.wait_ge(dma_sem, 32)
            gpsimd.sem_clear(dma_sem)
            gpsimd.sem_clear(vdone)
            gpsimd.sem_clear(dve_sem)

        @block.sync
        def _(sync):
            sync.dma_start(out=inpt[:], in_=inp).then_inc(dma_sem, 16)
            sync.wait_ge(vdone, 1)
            sync.dma_start(out=dist_out, in_=outt[:]).then_inc(dma_sem, 16)

        @block.vector
        def _(vector):
            # The DVE pipelines consecutive instructions: op N+1's reads can
            # overtake op N's writes, so RAW chains need same-engine
            # semaphore ordering (completion is in-order, so waiting on the
            # latest producer's ordinal covers everything before it).
            n = 0  # ordinal of the last emitted DVE op

            def op(inst):
                nonlocal n
                inst.then_inc(dve_sem, 1)
                n += 1
                return n

            def neq(d):
                i_lo = max(0, d - h)
                i_hi = min(d, r)
                L = i_hi - i_lo + 1
                # neq[i] = (ref[i-1] != hyp[d-i-1]); ref slot k = a[k-1],
                # hyp slot k = b[h-1-k] (slot h is pad)
                return op(vector.tensor_tensor(
                    out=nqs[d % 2][:, :, i_lo + 2 : i_hi + 3],
                    in0=reft[:, :, i_lo : i_lo + L],
                    in1=hypt[:, :, h - d + i_lo : h - d + i_lo + L],
                    op=AOT.not_equal,
                ))

            # guard init (independent of the input DMA)
            vector.wait_ge(dma_sem, 16)
            for rep in range(reps):
              if rep:
                  vector.wait_ge(dve_sem, n)  # previous rep fully done
              for k in range(3):
                op(vector.memset(v[k][:], BIG))
              op(vector.memset(v[0][:, :, 2:3], 0.0))  # D[0][0] on diagonal 0
              ord_prev_stt = None
              ord_neq = neq(1)
              for d in range(1, r + h + 1):
                i_lo = max(0, d - h)
                i_hi = min(d, r)
                prev = v[(d - 1) % 3]
                prev2 = v[(d - 2) % 3]
                cur = v[d % 3]
                w = slice(i_lo + 2, i_hi + 3)
                # wait: everything emitted before this point is complete
                vector.wait_ge(dve_sem, ord_prev_stt if d > 1 else ord_neq)
                op(vector.tensor_tensor(
                    out=mb[:, :, w],
                    in0=prev[:, :, w],
                    in1=prev[:, :, i_lo + 1 : i_hi + 2],
                    op=AOT.min,
                ))
                if d < r + h:
                    neq(d + 1)  # next diagonal's neq, hides behind this one
                ord_sb = op(vector.tensor_tensor(
                    out=sb[:, :, w],
                    in0=nqs[d % 2][:, :, w],
                    in1=prev2[:, :, i_lo + 1 : i_hi + 2],
                    op=AOT.add,
                ))
                vector.wait_ge(dve_sem, ord_sb)  # sb + mb complete
                ord_prev_stt = op(vector.scalar_tensor_tensor(
                    out=cur[:, :, w],
                    in0=mb[:, :, w],
                    scalar=1.0,
                    in1=sb[:, :, w],
                    op0=AOT.add,
                    op1=AOT.min,
                ))
            vector.wait_ge(dve_sem, ord_prev_stt)
            vector.tensor_copy(
                out=outt[:], in_=vf[:, :, r + 2 : r + 3]
            ).then_inc(vdone, 1)
    return nc


def _get_program(r=R, h=H, sstride=SSTRIDE, hstride=HSTRIDE, reps=1):
    key = (r, h, sstride, hstride, reps)
    if key not in _CACHE:
        _CACHE[key] = _build_program(r, h, sstride, hstride, reps)
    return _CACHE[key]


def _make_in_maps(ref_pair_f32, hyp_pair_f32, r=R, h=H, sstride=SSTRIDE, hstride=HSTRIDE):
    """ref_pair_f32: (B, r) float32 per-pair ref; hyp_pair_f32: (B, h)."""
    in_maps = []
    for c in range(NCORES):
        lo = c * BPC
        ra = ref_pair_f32[lo : lo + BPC]          # (512, r), b_local = g*128 + p
        ha = hyp_pair_f32[lo : lo + BPC]          # (512, h)
        ra = ra.reshape(SEG, P, r).transpose(1, 0, 2)   # -> [p, g, r]
        ha = ha.reshape(SEG, P, h).transpose(1, 0, 2)   # -> [p, g, h]
        arr = np.zeros((P, SEG, sstride + hstride), np.float32)
        arr[:, :, 1 : r + 1] = ra                        # ref slot k holds a[k-1]
        arr[:, :, sstride : sstride + h] = ha[:, :, ::-1]  # hyp slot k holds b[h-1-k]
        in_maps.append({"inp": arr})
    return in_maps


def _gather_dist(results):
    dist = np.empty(NCORES * BPC, np.float32)
    for c in range(NCORES):
        d = np.asarray(results[c]["dist"]).reshape(P, SEG)  # [p, g]
        dist[c * BPC : (c + 1) * BPC] = d.T.reshape(BPC)    # b_local = g*128 + p
    return dist


def run_device_dp(ref_pair_f32, hyp_pair_f32, r=R, h=H, trace=False):
    """Run the sharded edit-distance DP on the 8 NeuronCores.

    Returns (dist (B,) float32, BassKernelResults)."""
    from concourse.bass_utils import run_bass_kernel_spmd

    nc = _get_program(r, h)
    in_maps = _make_in_maps(ref_pair_f32, hyp_pair_f32, r, h)
    res = run_bass_kernel_spmd(
        nc, in_maps, list(range(NCORES)), trace=trace
    )
    return _gather_dist(res.results), res


def kernel(log_probs, ref, hyp):
    """Full-input entry point. log_probs (128,32) f32, ref (256,128) int64,
    hyp (288,128,32) int64 -> scalar float32 loss."""
    B = N * M
    refT = np.ascontiguousarray(np.asarray(ref).astype(np.float32).T)  # (N, R)
    hypT = np.ascontiguousarray(
        np.asarray(hyp).astype(np.float32).transpose(1, 2, 0)
    )  # (N, M, H)
    bidx = np.arange(B)
    ref_pair = refT[bidx // M]                 # (B, R)
    hyp_pair = hypT[bidx // M, bidx % M]       # (B, H)

    dist, _ = run_device_dp(ref_pair, hyp_pair)

    er = (dist / np.float32(R)).reshape(N, M)
    er = er - er.mean(axis=1, keepdims=True, dtype=np.float32)
    lp = np.asarray(log_probs).astype(np.float32)
    ex = np.exp(lp - lp.max(axis=1, keepdims=True))
    sm = ex / ex.sum(axis=1, keepdims=True, dtype=np.float32)
    return np.asarray((er * sm).mean(dtype=np.float32), dtype=np.float32)

